# revision 15
# baseline (speedup 1.0000x reference)
"""Student-t VQ soft-assignment (ClusteringLayer) on 8 Trainium2 NeuronCores.

q[b,k] = u / sum_k u,  u = 1/(1 + |x_b - c_k|^2)   (ALPHA = 1)

Strategy (data-parallel over batch, centroid table replicated):
  host: xT = x.T cast to bf16, sharded by batch into 8x [256, 8192];
        cmat = -2 * clusters.T (bf16); csq1 = 1 + |c_k|^2 (f32, from the
        bf16-rounded clusters so it is consistent with the matmul operand);
        xsq = per-row |x|^2 (f32, exact) laid out [128, 64] per core so
        column t holds the 128 partition values of row-tile t.
  core: per 128-row tile,
        PE  : m = -2 x.c^T via two bf16 matmuls (d split 2x128) -> PSUM
        DVE : one fused custom op: r = recip1NR(m + xsq + csq1),
              accum_out = row-sum(r)   (bit-flip seed + 1 Newton step)
        DVE : s_recip = recip_approx_fast(row sums)
        scale q = r * (1/s) -> bf16, split across engines per group:
          most tiles ACT Copy(scale=sr), a few on Pool normalize_recip,
          tail groups one DVE tensor_tensor with stride-0 sr broadcast.
        DMA : ONE output DMA per group (q tiles contiguous in SBUF),
              3D DRAM access pattern -> amortizes the ~625ns HWDGE issue
              cost that dominates per-tile DMA.
  host: concat shards, upcast bf16 -> f32.
"""

import os
from contextlib import ExitStack
from operator import add as _add

import numpy as np
import ml_dtypes

N_CORES = 8
B_FULL = 65536
B_CORE = B_FULL // N_CORES  # 8192
D = 256
K = 512
N_TILES = B_CORE // 128  # 64
POOL_TPG = int(os.environ.get("VQ_POOL_TPG", "4"))
SCHED = os.environ.get("VQ_SCHED", "fine")

# 1-NR bit-flip reciprocal constants (Chebyshev pair over [-4.5,-4])
SEED_SCALE = -0.23549792
NR_CONST = 2.0017324

LAST_EXEC_NS = None
LAST_RESULTS = None

_FUSED_OP = None
_NC_CACHE = None


def _ensure_ntff_hook():
    """This image's antenv lacks the tiny axon_hooks shim; synthesize it so
    BASS_TRACE=1 can capture an NTFF profile through libaxon_pjrt.so."""
    import sys
    import types
    try:
        import antenv.axon_hooks  # noqa: F401
        return
    except ImportError:
        pass
    try:
        import antenv
        mod = types.ModuleType("antenv.axon_hooks")
        mod._hook = None

        def set_axon_ntff_profile_hook(h):
            mod._hook = h

        def get_axon_ntff_profile_hook():
            return mod._hook

        mod.set_axon_ntff_profile_hook = set_axon_ntff_profile_hook
        mod.get_axon_ntff_profile_hook = get_axon_ntff_profile_hook
        sys.modules["antenv.axon_hooks"] = mod
        antenv.axon_hooks = mod
        from trn_agent_boot.trn_boot import _ntff_profile_via_ctypes
        set_axon_ntff_profile_hook(
            _ntff_profile_via_ctypes("/opt/axon/libaxon_pjrt.so"))
    except Exception:
        pass


def _register_fused_op():
    """Custom DVE op: out = recip1nr(in0 + s0 + in1), accum_out = row-sum(out).

    in0: PSUM m = -2 x.cT   s0: per-partition |x|^2   in1: broadcast 1+|c|^2.
    7 ALU stages + accumulator (fits the 8-slice budget).
    """
    global _FUSED_OP
    if _FUSED_OP is not None:
        return _FUSED_OP
    import concourse.dve_ops as dve_ops
    from concourse.dve_spec import (
        AluOp, Bin, C0, C1, C2, Spec, Src0, Src1, Zero, _has_src1, lower,
    )
    from concourse.dve_uop import DveOpSpec

    name = "VQ_RECIP1NR_BIAS_SUM"
    for op in dve_ops.OPS:
        if op.name == name:
            _FUSED_OP = op
            return op

    _m = (Src0 + C0) + Src1
    _n = Bin(AluOp.BITWISE_NOT, _m, _m)
    _y0 = _n * C1
    body = _y0 * (C2 - _m * _y0)

    def _ref(in0, in1, c0, c1, c2):
        m = (in0.astype(np.float32) + c0) + in1
        n = (~m.view(np.int32)).view(np.float32)
        y0 = n * c1
        y1 = y0 * (c2 - m * y0)
        return y1, y1.reshape(y1.shape[0], -1).sum(-1, keepdims=True)

    spec = Spec(body=body, accum=_add, accum_init=Zero, reference=_ref)
    row = max(dve_ops._SUB_OPCODE_FOR_NAME.values()) + 1
    shas = {}
    for ver in ("v3", "v4"):
        try:
            uops = lower(spec, ver=ver)
            shas[ver] = DveOpSpec(
                name=name, opcode=row, uops=uops, rd1_en=_has_src1(spec)
            ).sha(ver)
        except Exception:
            pass
    op = dve_ops.DveOp(name, spec, subdim=False, uops_sha=shas)
    dve_ops.OPS.append(op)
    dve_ops.CUSTOM_DVE_SPECS[name] = spec
    dve_ops._SUB_OPCODE_FOR_NAME[name] = row
    _FUSED_OP = op
    return op


def _view3(ap2d, size, width):
    """Reshape a [128, size*width] AP into a [128, size, width] 3-D view."""
    import concourse.bass as bass
    p = list(ap2d.ap[0])
    inner = ap2d.ap[1]
    assert inner[0] == 1 and inner[1] == size * width, (inner, size, width)
    return bass.AP(tensor=ap2d.tensor, offset=ap2d.offset,
                   ap=[p, [width, size], [1, width]])


def _bcast3(ap2d, size, width):
    """View a [128, size] AP as [128, size, width] via stride-0 broadcast."""
    import concourse.bass as bass
    p = list(ap2d.ap[0])
    inner = ap2d.ap[1]
    assert inner[0] == 1 and inner[1] == size, (inner, size)
    return bass.AP(tensor=ap2d.tensor, offset=ap2d.offset,
                   ap=[p, [1, size], [0, width]])


def _build_nc():
    global _NC_CACHE
    key = (POOL_TPG, SCHED)
    if _NC_CACHE is not None and _NC_CACHE[0] == key:
        return _NC_CACHE[1]
    import concourse.bass as bass
    import concourse.bacc as bacc
    import concourse.tile as tile
    import concourse.mybir as mybir

    op = _register_fused_op()
    BF = mybir.dt.bfloat16
    F32 = mybir.dt.float32
    ACT_COPY = mybir.ActivationFunctionType.Copy

    nc = bacc.Bacc("TRN2", target_bir_lowering=False, debug=False,
                   num_devices=N_CORES)
    # x packed [128, 2, B]: xt2[p, h, b] = x.T[h*128 + p, b] -> one input
    # DMA per group covers both contraction halves.
    xt = nc.dram_tensor("xt2", [128, 2 * B_CORE], BF,
                        kind="ExternalInput").ap()
    cm = nc.dram_tensor("cmat", [D, K], BF, kind="ExternalInput").ap()
    cs = nc.dram_tensor("csq1", [1, K], F32, kind="ExternalInput").ap()
    xq = nc.dram_tensor("xsq", [128, N_TILES], F32, kind="ExternalInput").ap()
    qo = nc.dram_tensor("qo", [B_CORE, K], BF, kind="ExternalOutput").ap()

    with tile.TileContext(nc) as tc, ExitStack() as ctx:
        const = ctx.enter_context(tc.tile_pool(name="const", bufs=1))
        xpool = ctx.enter_context(tc.tile_pool(name="x", bufs=6))
        rpool = ctx.enter_context(tc.tile_pool(name="r", bufs=4))
        qpool = ctx.enter_context(tc.tile_pool(name="q", bufs=5))
        spool = ctx.enter_context(tc.tile_pool(name="s", bufs=8))
        pm = ctx.enter_context(tc.tile_pool(name="pm", bufs=8, space="PSUM"))

        if SCHED == "fine":
            sizes = [1, 1, 2, 4] + [8] * 6 + [4, 2, 1, 1]
        elif SCHED == "grad":
            sizes = [2, 2, 4] + [8] * 6 + [4, 2, 2]
        else:
            sizes = [4] * 16
        assert sum(sizes) == N_TILES
        n_sched = len(sizes)

        def in_group_dma(dma, xab, gc, gw):
            src = bass.AP(tensor=xt.tensor, offset=gc,
                          ap=[[2 * B_CORE, 128], [B_CORE, 2], [1, gw]])
            dma(_view3(xab[:], 2, gw), src)

        # Startup ordering: group-0 input + centroid tiles split across the
        # two DMA queues so the first matmul's operands land ASAP; the
        # DVE-side broadcast const (csqb) leads the SWDGE queue.
        xab0 = xpool.tile([128, 2 * sizes[0] * 128], BF, tag="xab")
        in_group_dma(nc.sync.dma_start, xab0, 0, sizes[0] * 128)
        ct0 = const.tile([128, K], BF)
        nc.sync.dma_start(ct0[:], cm[0:128, :])
        # broadcast-load 1+|c|^2 across all 128 partitions
        csqb = const.tile([128, K], F32)
        cs_b = bass.AP(tensor=cs.tensor, offset=cs.offset,
                       ap=[[0, 128]] + [list(a) for a in cs.ap[1:]])
        nc.gpsimd.dma_start(csqb[:], cs_b)
        ct1 = const.tile([128, K], BF)
        nc.gpsimd.dma_start(ct1[:], cm[128:256, :])
        xsqt = const.tile([128, N_TILES], F32)
        nc.sync.dma_start(xsqt[:], xq[:, :])

        gc = 0
        for gi, size in enumerate(sizes):
            gw = size * 128
            lead = gi < 2            # fast-start groups: HWDGE input DMA
            tail_g = gi >= n_sched - 2
            if gi == 0:
                xab = xab0
            else:
                # keep the SWDGE queue clear near the tail (its DRAIN would
                # block the Pool scale ops that finish the kernel)
                on_sync = lead or gi >= n_sched - 3
                in_dma = nc.sync.dma_start if on_sync else nc.gpsimd.dma_start
                xab = xpool.tile([128, 2 * gw], BF, tag="xab")
                in_group_dma(in_dma, xab, gc, gw)
            xa = xab[:, 0:gw]
            xb = xab[:, gw:2 * gw]

            # how many of this group's tiles scale on Pool (normalize_recip)
            npool = 0
            if not tail_g and size >= 2:
                npool = min(POOL_TPG if size >= 8 else POOL_TPG - 1, size - 1)
                npool = max(npool, 0)
            nact = size - npool

            r_g = rpool.tile([128, size * K], F32, tag="r")
            q_g = qpool.tile([128, size * K], BF, tag="q")
            s_g = spool.tile([128, max(nact, 1)], F32, tag="s")
            sr_g = spool.tile([128, max(nact, 1)], F32, tag="sr")
            sp_cols = [spool.tile([128, 1], F32, tag=f"sp{j}",
                                  name=f"sp{j}")
                       for j in range(npool)]

            t0 = gc // 128
            for i in range(size):
                c0, c1 = i * 128, (i + 1) * 128
                pmm = pm.tile([128, K], F32)
                nc.tensor.matmul(pmm[:], xa[:, c0:c1], ct0[:],
                                 start=True, stop=False)
                nc.tensor.matmul(pmm[:], xb[:, c0:c1], ct1[:],
                                 start=False, stop=True)
                if i < nact:
                    acc = s_g[:, i:i + 1]
                else:
                    acc = sp_cols[i - nact][:]
                nc.vector._custom_dve(
                    op, out=r_g[:, i * K:(i + 1) * K], in0=pmm[:],
                    in1=csqb[:],
                    s0=xsqt[:, t0 + i:t0 + i + 1], s1=SEED_SCALE,
                    imm2=NR_CONST,
                    accum_out=acc,
                )

            nc.vector.reciprocal_approx_fast(out=sr_g[:], in_=s_g[:])
            for i in range(nact):
                nc.scalar.activation(q_g[:, i * K:(i + 1) * K],
                                     r_g[:, i * K:(i + 1) * K], ACT_COPY,
                                     bias=0.0, scale=sr_g[:, i:i + 1])
            for j in range(npool):
                i = nact + j
                nc.gpsimd.normalize_recip(
                    q_g[:, i * K:(i + 1) * K],
                    r_g[:, i * K:(i + 1) * K],
                    sp_cols[j][:])

            if gi == n_sched - 1:
                # split the final transfers so their latency (+ completion
                # semaphore) doesn't sit exposed at the end of the kernel
                half = 64
                for h in range(2):
                    out_ap = bass.AP(
                        tensor=qo.tensor, offset=(gc + h * half) * K,
                        ap=[[K, half], [128 * K, size], [1, K]])
                    nc.sync.dma_start(out_ap,
                                      _view3(q_g[h * half:h * half + half, :],
                                             size, K))
            else:
                # one output DMA for the whole group: [128, size, K] -> rows
                out_ap = bass.AP(tensor=qo.tensor, offset=gc * K,
                                 ap=[[K, 128], [128 * K, size], [1, K]])
                nc.sync.dma_start(out_ap, _view3(q_g[:], size, K))
            gc += gw

    nc.compile()
    _NC_CACHE = (key, nc)
    return nc


def kernel(x, clusters):
    """Full inputs in, full output out. Shards over 8 NeuronCores inside."""
    global LAST_EXEC_NS, LAST_RESULTS
    if os.environ.get("BASS_TRACE"):
        _ensure_ntff_hook()
    from concourse.bass_utils import run_bass_kernel_spmd

    x = np.asarray(x, dtype=np.float32)
    clusters = np.asarray(clusters, dtype=np.float32)

    # host-side layout prep: transpose + bf16 cast + shard
    xt = np.ascontiguousarray(x.T).astype(ml_dtypes.bfloat16)   # [256, 65536]
    cb = clusters.astype(ml_dtypes.bfloat16)                    # bf16 rounding
    cbf = cb.astype(np.float32)
    cmat = np.ascontiguousarray(cbf.T * -2.0).astype(ml_dtypes.bfloat16)
    csq1 = (1.0 + (cbf.astype(np.float64) ** 2).sum(1)).astype(np.float32)
    csq1 = np.ascontiguousarray(csq1[None, :])                  # [1, 512]
    xsq_full = np.einsum('bd,bd->b', x, x)                      # [65536] f32

    nc = _build_nc()
    in_maps = []
    for c in range(N_CORES):
        xc = xt[:, c * B_CORE:(c + 1) * B_CORE]                 # [256, 8192]
        shard = np.ascontiguousarray(
            xc.reshape(2, 128, B_CORE).transpose(1, 0, 2)
        ).reshape(128, 2 * B_CORE)                              # [128, 2*B]
        xsq_c = np.ascontiguousarray(
            xsq_full[c * B_CORE:(c + 1) * B_CORE]
            .reshape(N_TILES, 128).T.astype(np.float32))        # [128, 64]
        in_maps.append({"xt2": shard, "cmat": cmat, "csq1": csq1,
                        "xsq": xsq_c})

    res = run_bass_kernel_spmd(nc, in_maps, core_ids=list(range(N_CORES)))
    LAST_RESULTS = res
    LAST_EXEC_NS = res.exec_time_ns
    out = np.concatenate(
        [res.results[c]["qo"] for c in range(N_CORES)], axis=0)
    return out.astype(np.float32)


if __name__ == "__main__":
    rng = np.random.default_rng(0)
    x = rng.standard_normal((B_FULL, D), dtype=np.float32)
    c = rng.standard_normal((K, D), dtype=np.float32)
    q = kernel(x, c)
    print("out", q.shape, q.dtype, "row0 sum", q[0].sum())


# revision 16
# speedup vs baseline: 1.0178x; 1.0178x over previous
"""Student-t VQ soft-assignment (ClusteringLayer) on 8 Trainium2 NeuronCores.

q[b,k] = u / sum_k u,  u = 1/(1 + |x_b - c_k|^2)   (ALPHA = 1)

Strategy (data-parallel over batch, centroid table replicated):
  host: xT = x.T cast to bf16, sharded by batch into 8x [256, 8192];
        cmat = -2 * clusters.T (bf16); csq1 = 1 + |c_k|^2 (f32, from the
        bf16-rounded clusters so it is consistent with the matmul operand);
        xsq = per-row |x|^2 (f32, exact) laid out [128, 64] per core so
        column t holds the 128 partition values of row-tile t.
  core: per 128-row tile,
        PE  : m = -2 x.c^T via two bf16 matmuls (d split 2x128) -> PSUM
        DVE : one fused custom op: r = recip1NR(m + xsq + csq1),
              accum_out = row-sum(r)   (bit-flip seed + 1 Newton step)
        DVE : s_recip = recip_approx_fast(row sums)
        scale q = r * (1/s) -> bf16, split across engines per group:
          most tiles ACT Copy(scale=sr), a few on Pool normalize_recip,
          tail groups one DVE tensor_tensor with stride-0 sr broadcast.
        DMA : ONE output DMA per group (q tiles contiguous in SBUF),
              3D DRAM access pattern -> amortizes the ~625ns HWDGE issue
              cost that dominates per-tile DMA.
  host: concat shards, upcast bf16 -> f32.
"""

import os
from contextlib import ExitStack
from operator import add as _add

import numpy as np
import ml_dtypes

N_CORES = 8
B_FULL = 65536
B_CORE = B_FULL // N_CORES  # 8192
D = 256
K = 512
N_TILES = B_CORE // 128  # 64
POOL_TPG = int(os.environ.get("VQ_POOL_TPG", "4"))
SCHED = os.environ.get("VQ_SCHED", "fine")

# 1-NR bit-flip reciprocal constants (Chebyshev pair over [-4.5,-4])
SEED_SCALE = -0.23549792
NR_CONST = 2.0017324

LAST_EXEC_NS = None
LAST_RESULTS = None

_FUSED_OP = None
_NC_CACHE = None


def _ensure_ntff_hook():
    """This image's antenv lacks the tiny axon_hooks shim; synthesize it so
    BASS_TRACE=1 can capture an NTFF profile through libaxon_pjrt.so."""
    import sys
    import types
    try:
        import antenv.axon_hooks  # noqa: F401
        return
    except ImportError:
        pass
    try:
        import antenv
        mod = types.ModuleType("antenv.axon_hooks")
        mod._hook = None

        def set_axon_ntff_profile_hook(h):
            mod._hook = h

        def get_axon_ntff_profile_hook():
            return mod._hook

        mod.set_axon_ntff_profile_hook = set_axon_ntff_profile_hook
        mod.get_axon_ntff_profile_hook = get_axon_ntff_profile_hook
        sys.modules["antenv.axon_hooks"] = mod
        antenv.axon_hooks = mod
        from trn_agent_boot.trn_boot import _ntff_profile_via_ctypes
        set_axon_ntff_profile_hook(
            _ntff_profile_via_ctypes("/opt/axon/libaxon_pjrt.so"))
    except Exception:
        pass


def _register_fused_op():
    """Custom DVE op: out = recip1nr(in0 + s0 + in1), accum_out = row-sum(out).

    in0: PSUM m = -2 x.cT   s0: per-partition |x|^2   in1: broadcast 1+|c|^2.
    7 ALU stages + accumulator (fits the 8-slice budget).
    """
    global _FUSED_OP
    if _FUSED_OP is not None:
        return _FUSED_OP
    import concourse.dve_ops as dve_ops
    from concourse.dve_spec import (
        AluOp, Bin, C0, C1, C2, Spec, Src0, Src1, Zero, _has_src1, lower,
    )
    from concourse.dve_uop import DveOpSpec

    name = "VQ_RECIP1NR_BIAS_SUM"
    for op in dve_ops.OPS:
        if op.name == name:
            _FUSED_OP = op
            return op

    _m = (Src0 + C0) + Src1
    _n = Bin(AluOp.BITWISE_NOT, _m, _m)
    _y0 = _n * C1
    body = _y0 * (C2 - _m * _y0)

    def _ref(in0, in1, c0, c1, c2):
        m = (in0.astype(np.float32) + c0) + in1
        n = (~m.view(np.int32)).view(np.float32)
        y0 = n * c1
        y1 = y0 * (c2 - m * y0)
        return y1, y1.reshape(y1.shape[0], -1).sum(-1, keepdims=True)

    spec = Spec(body=body, accum=_add, accum_init=Zero, reference=_ref)
    row = max(dve_ops._SUB_OPCODE_FOR_NAME.values()) + 1
    shas = {}
    for ver in ("v3", "v4"):
        try:
            uops = lower(spec, ver=ver)
            shas[ver] = DveOpSpec(
                name=name, opcode=row, uops=uops, rd1_en=_has_src1(spec)
            ).sha(ver)
        except Exception:
            pass
    op = dve_ops.DveOp(name, spec, subdim=False, uops_sha=shas)
    dve_ops.OPS.append(op)
    dve_ops.CUSTOM_DVE_SPECS[name] = spec
    dve_ops._SUB_OPCODE_FOR_NAME[name] = row
    _FUSED_OP = op
    return op


def _view3(ap2d, size, width):
    """Reshape a [128, size*width] AP into a [128, size, width] 3-D view."""
    import concourse.bass as bass
    p = list(ap2d.ap[0])
    inner = ap2d.ap[1]
    assert inner[0] == 1 and inner[1] == size * width, (inner, size, width)
    return bass.AP(tensor=ap2d.tensor, offset=ap2d.offset,
                   ap=[p, [width, size], [1, width]])


def _bcast3(ap2d, size, width):
    """View a [128, size] AP as [128, size, width] via stride-0 broadcast."""
    import concourse.bass as bass
    p = list(ap2d.ap[0])
    inner = ap2d.ap[1]
    assert inner[0] == 1 and inner[1] == size, (inner, size)
    return bass.AP(tensor=ap2d.tensor, offset=ap2d.offset,
                   ap=[p, [1, size], [0, width]])


def _build_nc():
    global _NC_CACHE
    key = (POOL_TPG, SCHED)
    if _NC_CACHE is not None and _NC_CACHE[0] == key:
        return _NC_CACHE[1]
    import concourse.bass as bass
    import concourse.bacc as bacc
    import concourse.tile as tile
    import concourse.mybir as mybir

    op = _register_fused_op()
    BF = mybir.dt.bfloat16
    F32 = mybir.dt.float32
    ACT_COPY = mybir.ActivationFunctionType.Copy

    nc = bacc.Bacc("TRN2", target_bir_lowering=False, debug=False,
                   num_devices=N_CORES)
    # x packed [128, 2, B]: xt2[p, h, b] = x.T[h*128 + p, b] -> one input
    # DMA per group covers both contraction halves.
    xt = nc.dram_tensor("xt2", [128, 2 * B_CORE], BF,
                        kind="ExternalInput").ap()
    cm = nc.dram_tensor("cmat", [D, K], BF, kind="ExternalInput").ap()
    cs = nc.dram_tensor("csq1", [1, K], F32, kind="ExternalInput").ap()
    xq = nc.dram_tensor("xsq", [128, N_TILES], F32, kind="ExternalInput").ap()
    qo = nc.dram_tensor("qo", [B_CORE, K], BF, kind="ExternalOutput").ap()

    with tile.TileContext(nc) as tc, ExitStack() as ctx:
        const = ctx.enter_context(tc.tile_pool(name="const", bufs=1))
        xpool = ctx.enter_context(tc.tile_pool(name="x", bufs=6))
        rpool = ctx.enter_context(tc.tile_pool(name="r", bufs=4))
        qpool = ctx.enter_context(tc.tile_pool(name="q", bufs=5))
        spool = ctx.enter_context(tc.tile_pool(name="s", bufs=8))
        pm = ctx.enter_context(tc.tile_pool(name="pm", bufs=8, space="PSUM"))

        if SCHED == "fine":
            sizes = [1, 1, 2, 4] + [8] * 6 + [4, 2, 1, 1]
        elif SCHED == "grad":
            sizes = [2, 2, 4] + [8] * 6 + [4, 2, 2]
        else:
            sizes = [4] * 16
        assert sum(sizes) == N_TILES
        n_sched = len(sizes)

        def in_group_dma(dma, xab, gc, gw):
            src = bass.AP(tensor=xt.tensor, offset=gc,
                          ap=[[2 * B_CORE, 128], [B_CORE, 2], [1, gw]])
            dma(_view3(xab[:], 2, gw), src)

        # Startup ordering: group-0 input + centroid tiles split across the
        # two DMA queues so the first matmul's operands land ASAP; the
        # DVE-side broadcast const (csqb) leads the SWDGE queue.
        xab0 = xpool.tile([128, 2 * sizes[0] * 128], BF, tag="xab")
        in_group_dma(nc.sync.dma_start, xab0, 0, sizes[0] * 128)
        ct0 = const.tile([128, K], BF)
        nc.sync.dma_start(ct0[:], cm[0:128, :])
        # broadcast-load 1+|c|^2 across all 128 partitions
        csqb = const.tile([128, K], F32)
        cs_b = bass.AP(tensor=cs.tensor, offset=cs.offset,
                       ap=[[0, 128]] + [list(a) for a in cs.ap[1:]])
        nc.gpsimd.dma_start(csqb[:], cs_b)
        ct1 = const.tile([128, K], BF)
        nc.gpsimd.dma_start(ct1[:], cm[128:256, :])
        xsqt = const.tile([128, N_TILES], F32)
        nc.sync.dma_start(xsqt[:], xq[:, :])

        gc = 0
        for gi, size in enumerate(sizes):
            gw = size * 128
            lead = gi < 2            # fast-start groups: HWDGE input DMA
            tail_g = gi >= n_sched - 2
            if gi == 0:
                xab = xab0
            else:
                # keep the SWDGE queue clear near the tail (its DRAIN would
                # block the Pool scale ops that finish the kernel)
                on_sync = lead or gi >= n_sched - 3
                in_dma = nc.sync.dma_start if on_sync else nc.gpsimd.dma_start
                xab = xpool.tile([128, 2 * gw], BF, tag="xab")
                in_group_dma(in_dma, xab, gc, gw)
            xa = xab[:, 0:gw]
            xb = xab[:, gw:2 * gw]

            # how many of this group's tiles scale on Pool (normalize_recip)
            npool = 0
            if not tail_g and size >= 2:
                npool = min(POOL_TPG if size >= 8 else POOL_TPG - 1, size - 1)
                npool = max(npool, 0)
            nact = size - npool

            r_g = rpool.tile([128, size * K], F32, tag="r")
            q_g = qpool.tile([128, size * K], BF, tag="q")
            s_g = spool.tile([128, max(nact, 1)], F32, tag="s")
            sr_g = spool.tile([128, max(nact, 1)], F32, tag="sr")
            sp_cols = [spool.tile([128, 1], F32, tag=f"sp{j}",
                                  name=f"sp{j}")
                       for j in range(npool)]

            t0 = gc // 128
            for i in range(size):
                c0, c1 = i * 128, (i + 1) * 128
                pmm = pm.tile([128, K], F32)
                nc.tensor.matmul(pmm[:], xa[:, c0:c1], ct0[:],
                                 start=True, stop=False)
                nc.tensor.matmul(pmm[:], xb[:, c0:c1], ct1[:],
                                 start=False, stop=True)
                if i < nact:
                    acc = s_g[:, i:i + 1]
                else:
                    acc = sp_cols[i - nact][:]
                nc.vector._custom_dve(
                    op, out=r_g[:, i * K:(i + 1) * K], in0=pmm[:],
                    in1=csqb[:],
                    s0=xsqt[:, t0 + i:t0 + i + 1], s1=SEED_SCALE,
                    imm2=NR_CONST,
                    accum_out=acc,
                )

            nc.vector.reciprocal_approx_fast(out=sr_g[:], in_=s_g[:])
            for i in range(nact):
                nc.scalar.activation(q_g[:, i * K:(i + 1) * K],
                                     r_g[:, i * K:(i + 1) * K], ACT_COPY,
                                     bias=0.0, scale=sr_g[:, i:i + 1])
            for j in range(npool):
                i = nact + j
                nc.gpsimd.normalize_recip(
                    q_g[:, i * K:(i + 1) * K],
                    r_g[:, i * K:(i + 1) * K],
                    sp_cols[j][:])

            if tail_g:
                # split the final transfers so their latency (+ completion
                # semaphore) doesn't sit exposed at the end of the kernel
                half = 64
                for h in range(2):
                    out_ap = bass.AP(
                        tensor=qo.tensor, offset=(gc + h * half) * K,
                        ap=[[K, half], [128 * K, size], [1, K]])
                    nc.sync.dma_start(out_ap,
                                      _view3(q_g[h * half:h * half + half, :],
                                             size, K))
            else:
                # one output DMA for the whole group: [128, size, K] -> rows
                out_ap = bass.AP(tensor=qo.tensor, offset=gc * K,
                                 ap=[[K, 128], [128 * K, size], [1, K]])
                nc.sync.dma_start(out_ap, _view3(q_g[:], size, K))
            gc += gw

    nc.compile()
    _NC_CACHE = (key, nc)
    return nc


def kernel(x, clusters):
    """Full inputs in, full output out. Shards over 8 NeuronCores inside."""
    global LAST_EXEC_NS, LAST_RESULTS
    if os.environ.get("BASS_TRACE"):
        _ensure_ntff_hook()
    from concourse.bass_utils import run_bass_kernel_spmd

    x = np.asarray(x, dtype=np.float32)
    clusters = np.asarray(clusters, dtype=np.float32)

    # host-side layout prep: transpose + bf16 cast + shard
    xt = np.ascontiguousarray(x.T).astype(ml_dtypes.bfloat16)   # [256, 65536]
    cb = clusters.astype(ml_dtypes.bfloat16)                    # bf16 rounding
    cbf = cb.astype(np.float32)
    cmat = np.ascontiguousarray(cbf.T * -2.0).astype(ml_dtypes.bfloat16)
    csq1 = (1.0 + (cbf.astype(np.float64) ** 2).sum(1)).astype(np.float32)
    csq1 = np.ascontiguousarray(csq1[None, :])                  # [1, 512]
    xsq_full = np.einsum('bd,bd->b', x, x)                      # [65536] f32

    nc = _build_nc()
    in_maps = []
    for c in range(N_CORES):
        xc = xt[:, c * B_CORE:(c + 1) * B_CORE]                 # [256, 8192]
        shard = np.ascontiguousarray(
            xc.reshape(2, 128, B_CORE).transpose(1, 0, 2)
        ).reshape(128, 2 * B_CORE)                              # [128, 2*B]
        xsq_c = np.ascontiguousarray(
            xsq_full[c * B_CORE:(c + 1) * B_CORE]
            .reshape(N_TILES, 128).T.astype(np.float32))        # [128, 64]
        in_maps.append({"xt2": shard, "cmat": cmat, "csq1": csq1,
                        "xsq": xsq_c})

    res = run_bass_kernel_spmd(nc, in_maps, core_ids=list(range(N_CORES)))
    LAST_RESULTS = res
    LAST_EXEC_NS = res.exec_time_ns
    out = np.concatenate(
        [res.results[c]["qo"] for c in range(N_CORES)], axis=0)
    return out.astype(np.float32)


if __name__ == "__main__":
    rng = np.random.default_rng(0)
    x = rng.standard_normal((B_FULL, D), dtype=np.float32)
    c = rng.standard_normal((K, D), dtype=np.float32)
    q = kernel(x, c)
    print("out", q.shape, q.dtype, "row0 sum", q[0].sum())
